# revision 19
# baseline (speedup 1.0000x reference)
"""Trainium2 Bass kernel for a single-layer MHA + FFN transformer block.

Reference computation (x: [1, 4096, 768], 12 heads, dff=3072):
    qkv = (x @ w_qkv + b_qkv)  -> q, k, v
    scores = q k^T / sqrt(768) ; wei = softmax(scores)
    attn = wei @ v  (concat heads)
    h = gelu(attn @ w_ff1 + b_ff1)
    out = h @ w_ff2 + b_ff2

Sharding: sequence-parallel over the 4096 tokens across 8 NeuronCores
(512 rows each). Every core computes q/k/v for its own rows; k/v blocks
are exchanged with three pipelined bf16 AllGathers (head-pair 0, pairs
1-2, pairs 3-5) so attention on pair p starts as soon as its slice of
k/v has arrived instead of waiting for one monolithic collective.

Precision: weights and x are cast to bf16 on the host (the 1/sqrt(d)
score scale is folded into the q columns of w_qkv); all matmuls run
bf16 with fp32 PSUM accumulation. Softmax exp is split between the
scalar engine (table exp) and the vector engine (a fused custom-DVE
cubic, exact to ~0.5% on the bounded logits), so the two engines share
the 25M-exponential bottleneck. The softmax denominator rides along as
a ones-column in the packed v tiles; normalization uses the 2-op
approximate reciprocal plus a single K=2 broadcast matmul per pair.
"""

import json as _json
import math

import numpy as np
import ml_dtypes

import concourse.bass as bass
import concourse.mybir as mybir
import concourse.tile as tile
from concourse.bass_utils import run_bass_kernel_spmd

# ---------------------------------------------------------------------------
# Workaround: the pinned walrus build only supports ONE embedded semaphore
# wait per instruction, but Tile's sem assigner attaches several. Split the
# excess onto standalone EventSemaphore instructions (pure waits) inserted
# just before the over-subscribed instruction (same engine => same program
# order, identical semantics).
# ---------------------------------------------------------------------------
_MAX_WAITS = 1
_ctr = [0]
if not getattr(bass.Bass, "_multiwait_patched", False):
    _orig_to_json_bytes = bass.Bass.to_json_bytes

    def _split_multiwait_json_bytes(self):
        bir = _json.loads(_orig_to_json_bytes(self))
        for f in bir["functions"]:
            for b in f["blocks"]:
                new_insts = []
                for inst in b["instructions"]:
                    si = inst.get("sync_info")
                    waits = si.get("on_wait", []) if si else []
                    if len(waits) > _MAX_WAITS:
                        excess, keep = waits[:-_MAX_WAITS], waits[-_MAX_WAITS:]
                        for k in range(0, len(excess), _MAX_WAITS):
                            _ctr[0] += 1
                            new_insts.append({
                                "debug": inst.get("debug", 0),
                                "engine": inst["engine"],
                                "ins": [], "outs": [],
                                "name": "I-waitsplit-%d" % _ctr[0],
                                "opcode": "EventSemaphore",
                                "sync_info": {"on_update": [],
                                              "on_wait": excess[k:k + _MAX_WAITS]},
                            })
                        si["on_wait"] = keep
                    new_insts.append(inst)
                b["instructions"] = new_insts
        return _json.dumps(bir).encode()

    bass.Bass.to_json_bytes = _split_multiwait_json_bytes
    bass.Bass._multiwait_patched = True



F32 = mybir.dt.float32
F32R = mybir.dt.float32r
BF16 = mybir.dt.bfloat16
AFT = mybir.ActivationFunctionType

R = 8          # cores
T = 4096       # sequence length
TL = T // R    # rows per core (512)
D = 768
H = 12
HD = D // H    # 64
DFF = 4 * D    # 3072
P = 128
NDT = D // P   # 6 d-tiles
NTT = TL // P  # 4 local t-tiles
NFT = DFF // P  # 24 dff tiles
NPAIR = H // 2  # 6 head pairs
SCALE = 1.0 / math.sqrt(D)

PW = 2 * (HD + 1)          # 130: padded v width per pair ([v_h|1] x 2)
K_ELEMS = P * TL           # 65536 per-pair k^T payload
V_ELEMS = TL * PW          # 66560 per-pair padded-v payload
PAIR_ELEMS = K_ELEMS + V_ELEMS
AG_GROUPS = [[0], [1, 2], [3, 4, 5]]   # pipelined AllGather batches

# exp on [-0.8, 0.8] as ((s+K1)*s + K2)*(s*K3) + 1, rel err <= 0.46%
EXP_K1 = 3.3446521216989074
EXP_K2 = 6.493501417829298
EXP_K3 = 0.15487538281525948
# chunks (index mod 7) routed to the vector-engine exp; rest use scalar
EXP_DVE_RESIDUES = (1, 3, 5)

_NC_CACHE = {}


# ---------------------------------------------------------------------------
# Custom fused DVE op: cubic exp approximation in ONE vector instruction.
# out = ((s + K1)*s + K2) * (s*K3) + 1  ==  1 + c1 s + c2 s^2 + c3 s^3
# Registered into concourse.dve_ops at import time (repo is read-only).
# ---------------------------------------------------------------------------
def _register_exp3():
    from concourse import dve_ops as dops
    from concourse.dve_spec import Spec, Src0, C0, C1, C2, One, lower
    from concourse.dve_uop import DveOpSpec
    from concourse.bass_utils import dve_ver_for

    name = "EXP3_ANT"
    for op in dops.OPS:
        if op.name == name:
            return op

    def _ref(in0, in1, s0, s1, imm2):
        return (((in0 + s0) * in0 + s1) * (in0 * imm2) + 1.0).astype(np.float32)

    spec = Spec(body=((Src0 + C0) * Src0 + C1) * (Src0 * C2) + One,
                reference=_ref)
    row = dops._CUSTOM_DVE_ROW_BASE + len(dops.OPS)
    assert row < 0x20
    dops._SUB_OPCODE_FOR_NAME[name] = row
    shas = {}
    for ver in ("v3", "v4"):
        try:
            s = DveOpSpec(name=name, opcode=row, uops=lower(spec, ver=ver),
                          rd1_en=False)
            shas[ver] = s.sha(ver)
        except Exception:
            pass
    assert shas, "EXP3_ANT failed to lower for every DveVer"
    op = dops.DveOp(name, spec, subdim=False, uops_sha=shas)
    dops.OPS.append(op)
    return op


EXP3 = _register_exp3()


def _build_nc():
    nc = bass.Bass(num_devices=R)
    x = nc.declare_dram_parameter("x", [TL, D], F32, isOutput=False)
    w_qkv = nc.declare_dram_parameter("w_qkv", [D, 3 * D], BF16, isOutput=False)
    b_qkv = nc.declare_dram_parameter("b_qkv", [3 * D], F32, isOutput=False)
    w_ff1 = nc.declare_dram_parameter("w_ff1", [D, DFF], BF16, isOutput=False)
    b_ff1 = nc.declare_dram_parameter("b_ff1", [DFF], F32, isOutput=False)
    w_ff2 = nc.declare_dram_parameter("w_ff2", [DFF, D], BF16, isOutput=False)
    b_ff2 = nc.declare_dram_parameter("b_ff2", [D], F32, isOutput=False)
    y = nc.declare_dram_parameter("y", [TL, D], F32, isOutput=True)

    from contextlib import ExitStack
    from concourse.masks import make_identity

    with tile.TileContext(nc) as tc, ExitStack() as top:
        const = top.enter_context(tc.tile_pool(name="const", bufs=1))
        dramp = top.enter_context(tc.tile_pool(name="dramp", bufs=1, space="DRAM"))
        persist = top.enter_context(tc.tile_pool(name="persist", bufs=1))

        ident = const.tile([P, P], F32, name="ident")
        make_identity(nc, ident)
        ones_dram = nc.inline_tensor(np.ones((1, P), np.float32), name="ones_const")
        ones_row = const.tile([1, P], F32R, name="ones_row")
        nc.sync.dma_start(ones_row[:], ones_dram.ap().bitcast(F32R))
        bq_sb = const.tile([P, 3 * D // P], F32, name="bq_sb")
        nc.sync.dma_start(bq_sb[:], b_qkv.ap().rearrange("(o p) -> p o", p=P))
        bv_sb = const.tile([1, D], F32R, name="bv_sb")
        nc.sync.dma_start(bv_sb[:], b_qkv.ap()[None, 2 * D:3 * D].bitcast(F32R))
        b1_sb = const.tile([P, NFT], F32, name="b1_sb")
        nc.sync.dma_start(b1_sb[:], b_ff1.ap().rearrange("(o p) -> p o", p=P))
        b2_sb = const.tile([1, D], F32R, name="b2_sb")
        nc.sync.dma_start(b2_sb[:], b_ff2.ap()[None, :].bitcast(F32R))

        # Tiny collective issued immediately: absorbs the rank barrier and
        # ncfw warm-up (~30us) concurrently with the projection phase so the
        # real AllGathers start with only per-op latency.
        warm_in = dramp.tile([64], BF16, name="warm_in")
        warm_out = dramp.tile([R * 64], BF16, addr_space="Shared",
                              name="warm_out")
        nc.vector.memset(warm_sb := const.tile([1, 64], BF16, name="warm_sb"), 0.0)
        nc.scalar.dma_start(warm_in[:].rearrange("(a b) -> a b", a=1), warm_sb[:])
        nc.gpsimd.collective_compute(
            "AllGather", mybir.AluOpType.bypass,
            replica_groups=[list(range(R))],
            ins=[warm_in[:]], outs=[warm_out[:]],
        )

        ag_ins, ag_outs = [], []
        for gi, grp in enumerate(AG_GROUPS):
            n = len(grp) * PAIR_ELEMS
            ag_ins.append(dramp.tile([n], BF16, name=f"ag_in{gi}"))
            ag_outs.append(dramp.tile([R * n], BF16, addr_space="Shared",
                                      name=f"ag_out{gi}"))

        attnT = [persist.tile([P, TL], BF16, name=f"attnT{i}") for i in range(NDT)]
        hTp = top.enter_context(tc.tile_pool(name="hTp", bufs=1))
        hT = [hTp.tile([P, TL], BF16, name=f"hT{f}") for f in range(NFT)]
        # FFN weight pools at top scope so their loads can prefetch during
        # the attention phase.
        w1p = top.enter_context(tc.tile_pool(name="w1p", bufs=3))
        w2p = top.enter_context(tc.tile_pool(name="w2p", bufs=4))

        kv_scope = top.enter_context(ExitStack())
        kvp = kv_scope.enter_context(tc.tile_pool(name="kvp", bufs=1))
        qT = [kvp.tile([P, TL], BF16, name=f"qT{p}") for p in range(NPAIR)]
        kT_loc = [kvp.tile([P, TL], BF16, name=f"kTl{p}") for p in range(NPAIR)]
        # padded v in two half-tiles per t-tile (pairs 0-2 | pairs 3-5) so
        # group-0 payload DMAs don't wait on the second half's projection
        v_half = [[kvp.tile([P, 3 * PW], BF16, name=f"vp{t}_{h}")
                   for h in range(2)] for t in range(NTT)]
        kTf = [[kvp.tile([P, TL], BF16, name=f"kTf{p}_{r}") for r in range(R)]
               for p in range(NPAIR)]
        vf = [[kvp.tile([P, NTT * PW], BF16, name=f"vf{p}_{r}") for r in range(R)]
              for p in range(NPAIR)]

        # ------------------------------------------------------------------
        # Phase 1: x -> x^T, QKV projections, pipelined k/v AllGathers
        # ------------------------------------------------------------------
        with ExitStack() as ph1:
            xp = ph1.enter_context(tc.tile_pool(name="xp", bufs=1))
            wqp = ph1.enter_context(tc.tile_pool(name="wqp", bufs=3))
            psT = ph1.enter_context(tc.tile_pool(name="psT", bufs=2, space="PSUM"))
            psQ = ph1.enter_context(tc.tile_pool(name="psQ", bufs=2, space="PSUM"))

            x_sb = [xp.tile([P, D], F32, name=f"x{t}") for t in range(NTT)]
            for t in range(NTT):
                nc.sync.dma_start(x_sb[t][:], x.ap()[P * t:P * (t + 1), :])

            xT = [xp.tile([P, TL], BF16, name=f"xT{d}") for d in range(NDT)]
            for dt_ in range(NDT):
                for tt in range(NTT):
                    pst = psT.tile([P, P], F32, tag="pst", name="pst")
                    nc.tensor.transpose(
                        pst[:], x_sb[tt][:, P * dt_:P * (dt_ + 1)], ident[:])
                    if (dt_ * NTT + tt) % 2 == 0:
                        nc.vector.tensor_copy(
                            xT[dt_][:, P * tt:P * (tt + 1)], pst[:])
                    else:
                        nc.scalar.copy(
                            xT[dt_][:, P * tt:P * (tt + 1)], pst[:])

            w_qkv_v = w_qkv.ap().rearrange("(o p) j -> p o j", p=P)

            def proj_jt(jt, out_tile, add_engine, dma_engine):
                """qkv^T tile for channel block jt: out[j, t] = W[:,j]^T x^T + b."""
                wq = wqp.tile([P, NDT, P], BF16, tag="wq", name="wq")
                dma_engine.dma_start(wq[:], w_qkv_v[:, :, P * jt:P * (jt + 1)])
                ps = psQ.tile([P, TL], F32, tag="psq", name="psq")
                for d_ in range(NDT):
                    nc.tensor.matmul(ps[:], wq[:, d_, :], xT[d_][:],
                                     start=(d_ == 0), stop=(d_ == NDT - 1))
                if add_engine == "vector":
                    nc.vector.tensor_scalar_add(out_tile[:], ps[:],
                                                bq_sb[:, jt:jt + 1])
                else:
                    nc.scalar.activation(out_tile[:], ps[:], AFT.Identity,
                                         bias=bq_sb[:, jt:jt + 1])

            def proj_v_half(o2):
                """v rows for heads [6*o2, 6*o2+6) into padded v_half tiles."""
                sl = slice(384 * o2, 384 * (o2 + 1))
                for tt in range(NTT):
                    ps = psQ.tile([P, TL], F32, tag="psq", name="psq")
                    for d_ in range(NDT):
                        nc.tensor.matmul(ps[:, :384],
                                         xT[d_][:, P * tt:P * (tt + 1)],
                                         wv[:, d_, sl],
                                         start=(d_ == 0), stop=False)
                    nc.tensor.matmul(ps[:, :384], ones_row[:], bv_sb[:, sl],
                                     start=False, stop=True)
                    vdst = v_half[tt][o2].rearrange("p (h e) -> p h e", e=HD + 1)
                    nc.vector.tensor_copy(
                        vdst[:, :, 0:HD],
                        ps[:, :384].rearrange("p (h e) -> p h e", e=HD))
                    nc.vector.memset(vdst[:, :, HD:HD + 1], 1.0)

            def stage_pair(gi, pi, p_):
                """Copy pair p_'s k/v payload into group gi's AG input."""
                off = pi * PAIR_ELEMS
                ag_k = ag_ins[gi][off:off + K_ELEMS].rearrange(
                    "(a b) -> a b", b=TL)
                nc.scalar.dma_start(ag_k[:, :], kT_loc[p_][:])
                ag_v = ag_ins[gi][off + K_ELEMS:off + PAIR_ELEMS].rearrange(
                    "(t c) -> t c", c=PW)
                half, pp = divmod(p_, 3)
                for tt in range(NTT):
                    nc.scalar.dma_start(
                        ag_v[P * tt:P * (tt + 1), :],
                        v_half[tt][half][:, PW * pp:PW * (pp + 1)])

            def kick(gi):
                nc.gpsimd.collective_compute(
                    "AllGather", mybir.AluOpType.bypass,
                    replica_groups=[list(range(R))],
                    ins=[ag_ins[gi][:]], outs=[ag_outs[gi][:]],
                )

            wv = xp.tile([P, NDT, D], BF16, name="wv")
            nc.scalar.dma_start(wv[:], w_qkv_v[:, :, 2 * D:3 * D])

            # group 0 (pair 0) as early as possible
            proj_jt(NDT + 0, kT_loc[0], "scalar", nc.sync)
            proj_v_half(0)
            stage_pair(0, 0, 0)
            kick(0)
            # group 1 (pairs 1-2)
            for p_ in (1, 2):
                proj_jt(NDT + p_, kT_loc[p_], "scalar", nc.sync)
                stage_pair(1, p_ - 1, p_)
            kick(1)
            # group 2 (pairs 3-5)
            proj_v_half(1)
            for p_ in (3, 4, 5):
                proj_jt(NDT + p_, kT_loc[p_], "scalar", nc.sync)
                stage_pair(2, p_ - 3, p_)
            kick(2)

            # q projections overlap the collectives
            for p_ in range(NPAIR):
                proj_jt(p_, qT[p_], "vector", nc.sync)

            # keep-warm filler: the PE would otherwise idle ~10-15us waiting
            # for the first AllGather, dropping the HAM clock gate back to
            # 1.2 GHz right as attention starts. Outputs are never read.
            for wi in range(28):
                psw = psQ.tile([P, TL], F32, tag="psq", name="psw")
                nc.tensor.matmul(psw[:], kT_loc[0][:, 0:P], xT[wi % NDT][:],
                                 start=True, stop=True)

            # AllGather returns: per (pair, rank) tiles so attention chunks
            # only wait on the slice they read; k and v on separate DMA rings.
            for gi, grp in enumerate(AG_GROUPS):
                n = len(grp) * PAIR_ELEMS
                ago = ag_outs[gi].rearrange("(r e) -> r e", e=n)
                for pi, p_ in enumerate(grp):
                    off = pi * PAIR_ELEMS
                    for r in range(R):
                        src_k = ago[r, off:off + K_ELEMS].rearrange(
                            "(a b) -> a b", b=TL)
                        nc.sync.dma_start(kTf[p_][r][:], src_k)
                        src_v = ago[r, off + K_ELEMS:off + PAIR_ELEMS].rearrange(
                            "(s pi2 c) -> pi2 s c", pi2=P, c=PW)
                        dst_v = vf[p_][r].rearrange("p (s c) -> p s c", c=PW)
                        nc.gpsimd.dma_start(dst_v[:], src_v)

        # ------------------------------------------------------------------
        # Phase 2: attention, one head pair at a time
        # ------------------------------------------------------------------
        with ExitStack() as ph2:
            scp = ph2.enter_context(tc.tile_pool(name="scp", bufs=2, space="PSUM"))
            accp = ph2.enter_context(tc.tile_pool(name="accp", bufs=4, space="PSUM"))
            weip = ph2.enter_context(tc.tile_pool(name="weip", bufs=3))
            tailp = ph2.enter_context(tc.tile_pool(name="tailp", bufs=2))

            for p_ in range(NPAIR):
                acc0 = accp.tile([HD + 1, TL], F32, tag="acc", name="acc0")
                acc1 = accp.tile([HD + 1, TL], F32, tag="acc", name="acc1")
                for c in range(R * NTT):
                    r, s = divmod(c, NTT)
                    kt = kTf[p_][r]
                    sc = scp.tile([P, 2 * TL], F32, tag="sc", name="sc")
                    nc.tensor.matmul(sc[:, 0:TL],
                                     kt[0:HD, P * s:P * (s + 1)],
                                     qT[p_][0:HD, :], start=True, stop=True)
                    nc.tensor.matmul(sc[:, TL:2 * TL],
                                     kt[HD:P, P * s:P * (s + 1)],
                                     qT[p_][HD:P, :], start=True, stop=True)
                    wei = weip.tile([P, 2 * TL], BF16, tag="wei", name="wei")
                    if c % 7 in EXP_DVE_RESIDUES:
                        nc.vector._custom_dve(EXP3, out=wei[:], in0=sc[:],
                                              s0=EXP_K1, s1=EXP_K2, imm2=EXP_K3)
                    else:
                        nc.scalar.activation(wei[:], sc[:], AFT.Exp)
                    vt = vf[p_][r]
                    nc.tensor.matmul(acc0[:],
                                     vt[:, PW * s:PW * s + HD + 1],
                                     wei[:, 0:TL],
                                     start=(c == 0), stop=(c == R * NTT - 1))
                    nc.tensor.matmul(acc1[:],
                                     vt[:, PW * s + HD + 1:PW * (s + 1)],
                                     wei[:, TL:2 * TL],
                                     start=(c == 0), stop=(c == R * NTT - 1))

                # softmax tail: den -> 1/den -> broadcast -> scale
                from concourse.dve_ops import (
                    RECIPROCAL_APPROX_FAST, RECIPROCAL_APPROX_NR,
                    RECIP_APPROX_FAST_CONSTS as _RC)
                for hh, acc in ((0, acc0), (1, acc1)):
                    # custom-DVE ops require equal partition bases on all
                    # operands; stage the denominator row at partition 0.
                    denc = tailp.tile([1, TL], F32, tag="denc", name="denc")
                    nc.vector.tensor_copy(denc[:], acc[HD:HD + 1, :])
                    den = tailp.tile([1, TL], F32R, tag="den", name="den")
                    scr = tailp.tile([1, TL], F32, tag="scr", name="scr")
                    nc.vector._custom_dve(
                        RECIPROCAL_APPROX_FAST, out=scr[:],
                        in0=denc[:], s0=_RC["s0"], s1=_RC["s1"],
                        imm2=_RC["imm2"])
                    nc.vector._custom_dve(
                        RECIPROCAL_APPROX_NR, out=den[:],
                        in0=denc[:], in1=scr[:], s0=2.0)
                    bc = scp.tile([HD, TL], F32, tag="sc", name="bc")
                    nc.tensor.matmul(bc[:], ones_row[:, 0:HD], den[:],
                                     start=True, stop=True)
                    recb = tailp.tile([HD, TL], F32, tag="recb", name="recb")
                    nc.vector.tensor_copy(recb[:], bc[:])
                    nc.vector.tensor_tensor(
                        attnT[p_][HD * hh:HD * (hh + 1), :], acc[0:HD, :],
                        recb[:], mybir.AluOpType.mult)

        kv_scope.close()

        # ------------------------------------------------------------------
        # Phase 3a: FFN1  h^T[f, t] = gelu(W1^T attn^T + b1)
        # ------------------------------------------------------------------
        w_ff1_v = w_ff1.ap().rearrange("(o p) f -> p o f", p=P)
        with ExitStack() as ph3:
            ps1 = ph3.enter_context(tc.tile_pool(name="ps1", bufs=2, space="PSUM"))
            for ft in range(NFT):
                w1 = w1p.tile([P, NDT, P], BF16, tag="w1", name="w1")
                nc.gpsimd.dma_start(w1[:], w_ff1_v[:, :, P * ft:P * (ft + 1)])
                ps = ps1.tile([P, TL], F32, tag="ps1t", name="ps1t")
                for d_ in range(NDT):
                    nc.tensor.matmul(ps[:], w1[:, d_, :], attnT[d_][:],
                                     start=(d_ == 0), stop=(d_ == NDT - 1))
                nc.scalar.activation(hT[ft][:], ps[:], AFT.Gelu,
                                     bias=b1_sb[:, ft:ft + 1])

        # ------------------------------------------------------------------
        # Phase 3b: FFN2  out[t, o] = h^T^T W2 + b2
        # ------------------------------------------------------------------
        w_ff2_v = w_ff2.ap().rearrange("(o p) d -> p o d", p=P)
        with ExitStack() as ph4:
            ps2 = ph4.enter_context(tc.tile_pool(name="ps2", bufs=1, space="PSUM"))
            outp = ph4.enter_context(tc.tile_pool(name="outp", bufs=1))
            acc2 = [ps2.tile([P, 384], F32, name=f"acc2_{g}") for g in range(8)]
            for ft in range(NFT):
                w2 = w2p.tile([P, D], BF16, tag="w2", name="w2")
                nc.gpsimd.dma_start(w2[:], w_ff2_v[:, ft, :])
                for tt in range(NTT):
                    for o2 in range(2):
                        g = tt * 2 + o2
                        nc.tensor.matmul(acc2[g][:],
                                         hT[ft][:, P * tt:P * (tt + 1)],
                                         w2[:, 384 * o2:384 * (o2 + 1)],
                                         start=(ft == 0), stop=False)
            out_sb = [outp.tile([P, D], F32, name=f"out{tt}") for tt in range(NTT)]
            for tt in range(NTT):
                for o2 in range(2):
                    g = tt * 2 + o2
                    sl = slice(384 * o2, 384 * (o2 + 1))
                    nc.tensor.matmul(acc2[g][:], ones_row[:], b2_sb[:, sl],
                                     start=False, stop=True)
                    nc.vector.tensor_copy(out_sb[tt][:, sl], acc2[g][:])
                nc.scalar.dma_start(y.ap()[P * tt:P * (tt + 1), :], out_sb[tt][:])

    # The Tile path never runs bacc's codegen_inst_isa_subclasses pass, so
    # custom-DVE ISA wrappers would serialize with empty instruction bytes
    # ("ISA wrong length" in walrus). Lower them in place here.
    import concourse.bass_isa as bass_isa
    for func in nc.m.functions:
        for blk in func.blocks:
            i = 0
            while i < len(blk.instructions):
                inst = blk.instructions[i]
                if isinstance(inst, bass_isa.InstCustomDveAnt):
                    lowered = mybir.codegen_inst_isa_one(inst, nc._state, nc.isa)
                    assert isinstance(lowered, list)
                    del nc.inst_map[inst.name]
                    blk.instructions[i:i + 1] = lowered
                    for li in lowered:
                        nc.inst_map[li.name] = li
                    i += len(lowered)
                else:
                    i += 1

    return nc


def _get_nc():
    if "nc" not in _NC_CACHE:
        _NC_CACHE["nc"] = _build_nc()
    return _NC_CACHE["nc"]


def run_sharded(inputs, **run_kwargs):
    """Run the SPMD kernel; returns (full_output [1,4096,768], BassKernelResults)."""
    x = np.ascontiguousarray(np.asarray(inputs["x"], dtype=np.float32))
    assert x.shape == (1, T, D), x.shape

    w_qkv = np.asarray(inputs["w_qkv"], dtype=np.float32).copy()
    b_qkv = np.asarray(inputs["b_qkv"], dtype=np.float32).copy()
    # fold the 1/sqrt(d) score scale into the q projection
    w_qkv[:, 0:D] *= SCALE
    b_qkv[0:D] *= SCALE

    common = {
        "w_qkv": np.ascontiguousarray(w_qkv.astype(ml_dtypes.bfloat16)),
        "b_qkv": np.ascontiguousarray(b_qkv),
        "w_ff1": np.ascontiguousarray(
            np.asarray(inputs["w_ff1"], dtype=np.float32).astype(ml_dtypes.bfloat16)),
        "b_ff1": np.ascontiguousarray(np.asarray(inputs["b_ff1"], dtype=np.float32)),
        "w_ff2": np.ascontiguousarray(
            np.asarray(inputs["w_ff2"], dtype=np.float32).astype(ml_dtypes.bfloat16)),
        "b_ff2": np.ascontiguousarray(np.asarray(inputs["b_ff2"], dtype=np.float32)),
    }
    in_maps = []
    for r in range(R):
        m = dict(common)
        m["x"] = np.ascontiguousarray(x[0, TL * r:TL * (r + 1), :])
        in_maps.append(m)
    nc = _get_nc()
    res = run_bass_kernel_spmd(nc, in_maps, core_ids=list(range(R)), **run_kwargs)
    out = np.concatenate([res.results[r]["y"] for r in range(R)], axis=0)
    return out.reshape(1, T, D), res


def kernel(**inputs):
    out, _ = run_sharded(inputs)
    return out


# revision 20
# speedup vs baseline: 1.1506x; 1.1506x over previous
"""Trainium2 Bass kernel for a single-layer MHA + FFN transformer block.

Reference computation (x: [1, 4096, 768], 12 heads, dff=3072):
    qkv = (x @ w_qkv + b_qkv)  -> q, k, v
    scores = q k^T / sqrt(768) ; wei = softmax(scores)
    attn = wei @ v  (concat heads)
    h = gelu(attn @ w_ff1 + b_ff1)
    out = h @ w_ff2 + b_ff2

Sharding: sequence-parallel over the 4096 tokens across 8 NeuronCores
(512 rows each). Every core computes q/k/v for its own rows; k/v blocks
are exchanged with three pipelined bf16 AllGathers (head-pair 0, pairs
1-2, pairs 3-5) so attention on pair p starts as soon as its slice of
k/v has arrived instead of waiting for one monolithic collective.

Precision: weights and x are cast to bf16 on the host (the 1/sqrt(d)
score scale is folded into the q columns of w_qkv); all matmuls run
bf16 with fp32 PSUM accumulation. Softmax exp is split between the
scalar engine (table exp) and the vector engine (a fused custom-DVE
cubic, exact to ~0.5% on the bounded logits), so the two engines share
the 25M-exponential bottleneck. The softmax denominator rides along as
a ones-column in the packed v tiles; normalization uses the 2-op
approximate reciprocal plus a single K=2 broadcast matmul per pair.
"""

import json as _json
import math

import numpy as np
import ml_dtypes

import concourse.bass as bass
import concourse.mybir as mybir
import concourse.tile as tile
from concourse.bass_utils import run_bass_kernel_spmd

# ---------------------------------------------------------------------------
# Workaround: the pinned walrus build only supports ONE embedded semaphore
# wait per instruction, but Tile's sem assigner attaches several. Split the
# excess onto standalone EventSemaphore instructions (pure waits) inserted
# just before the over-subscribed instruction (same engine => same program
# order, identical semantics).
# ---------------------------------------------------------------------------
_MAX_WAITS = 1
_ctr = [0]
if not getattr(bass.Bass, "_multiwait_patched", False):
    _orig_to_json_bytes = bass.Bass.to_json_bytes

    def _split_multiwait_json_bytes(self):
        bir = _json.loads(_orig_to_json_bytes(self))
        for f in bir["functions"]:
            for b in f["blocks"]:
                new_insts = []
                for inst in b["instructions"]:
                    si = inst.get("sync_info")
                    waits = si.get("on_wait", []) if si else []
                    if len(waits) > _MAX_WAITS:
                        excess, keep = waits[:-_MAX_WAITS], waits[-_MAX_WAITS:]
                        for k in range(0, len(excess), _MAX_WAITS):
                            _ctr[0] += 1
                            new_insts.append({
                                "debug": inst.get("debug", 0),
                                "engine": inst["engine"],
                                "ins": [], "outs": [],
                                "name": "I-waitsplit-%d" % _ctr[0],
                                "opcode": "EventSemaphore",
                                "sync_info": {"on_update": [],
                                              "on_wait": excess[k:k + _MAX_WAITS]},
                            })
                        si["on_wait"] = keep
                    new_insts.append(inst)
                b["instructions"] = new_insts
        return _json.dumps(bir).encode()

    bass.Bass.to_json_bytes = _split_multiwait_json_bytes
    bass.Bass._multiwait_patched = True



F32 = mybir.dt.float32
F32R = mybir.dt.float32r
BF16 = mybir.dt.bfloat16
AFT = mybir.ActivationFunctionType

R = 8          # cores
T = 4096       # sequence length
TL = T // R    # rows per core (512)
D = 768
H = 12
HD = D // H    # 64
DFF = 4 * D    # 3072
P = 128
NDT = D // P   # 6 d-tiles
NTT = TL // P  # 4 local t-tiles
NFT = DFF // P  # 24 dff tiles
NPAIR = H // 2  # 6 head pairs
SCALE = 1.0 / math.sqrt(D)

PW = 2 * (HD + 1)          # 130: padded v width per pair ([v_h|1] x 2)
K_ELEMS = P * TL           # 65536 per-pair k^T payload
V_ELEMS = TL * PW          # 66560 per-pair padded-v payload
PAIR_ELEMS = K_ELEMS + V_ELEMS
AG_GROUPS = [[0], [1, 2], [3, 4, 5]]   # pipelined AllGather batches

# exp on [-0.8, 0.8] as ((s+K1)*s + K2)*(s*K3) + 1, rel err <= 0.46%
EXP_K1 = 3.3446521216989074
EXP_K2 = 6.493501417829298
EXP_K3 = 0.15487538281525948
# chunks (index mod 7) routed to the vector-engine exp; rest use scalar
EXP_DVE_RESIDUES = (1, 3, 5)

_NC_CACHE = {}


# ---------------------------------------------------------------------------
# Custom fused DVE op: cubic exp approximation in ONE vector instruction.
# out = ((s + K1)*s + K2) * (s*K3) + 1  ==  1 + c1 s + c2 s^2 + c3 s^3
# Registered into concourse.dve_ops at import time (repo is read-only).
# ---------------------------------------------------------------------------
def _register_exp3():
    from concourse import dve_ops as dops
    from concourse.dve_spec import Spec, Src0, C0, C1, C2, One, lower
    from concourse.dve_uop import DveOpSpec
    from concourse.bass_utils import dve_ver_for

    name = "EXP3_ANT"
    for op in dops.OPS:
        if op.name == name:
            return op

    def _ref(in0, in1, s0, s1, imm2):
        return (((in0 + s0) * in0 + s1) * (in0 * imm2) + 1.0).astype(np.float32)

    spec = Spec(body=((Src0 + C0) * Src0 + C1) * (Src0 * C2) + One,
                reference=_ref)
    row = dops._CUSTOM_DVE_ROW_BASE + len(dops.OPS)
    assert row < 0x20
    dops._SUB_OPCODE_FOR_NAME[name] = row
    shas = {}
    for ver in ("v3", "v4"):
        try:
            s = DveOpSpec(name=name, opcode=row, uops=lower(spec, ver=ver),
                          rd1_en=False)
            shas[ver] = s.sha(ver)
        except Exception:
            pass
    assert shas, "EXP3_ANT failed to lower for every DveVer"
    op = dops.DveOp(name, spec, subdim=False, uops_sha=shas)
    dops.OPS.append(op)
    return op


EXP3 = _register_exp3()


def _build_nc():
    nc = bass.Bass(num_devices=R)
    x = nc.declare_dram_parameter("x", [TL, D], F32, isOutput=False)
    w_qkv = nc.declare_dram_parameter("w_qkv", [D, 3 * D], BF16, isOutput=False)
    b_qkv = nc.declare_dram_parameter("b_qkv", [3 * D], F32, isOutput=False)
    w_ff1 = nc.declare_dram_parameter("w_ff1", [D, DFF], BF16, isOutput=False)
    b_ff1 = nc.declare_dram_parameter("b_ff1", [DFF], F32, isOutput=False)
    w_ff2 = nc.declare_dram_parameter("w_ff2", [DFF, D], BF16, isOutput=False)
    b_ff2 = nc.declare_dram_parameter("b_ff2", [D], F32, isOutput=False)
    y = nc.declare_dram_parameter("y", [TL, D], F32, isOutput=True)

    from contextlib import ExitStack
    from concourse.masks import make_identity

    with tile.TileContext(nc) as tc, ExitStack() as top:
        const = top.enter_context(tc.tile_pool(name="const", bufs=1))
        dramp = top.enter_context(tc.tile_pool(name="dramp", bufs=1, space="DRAM"))
        persist = top.enter_context(tc.tile_pool(name="persist", bufs=1))

        ident = const.tile([P, P], F32, name="ident")
        make_identity(nc, ident)
        ones_dram = nc.inline_tensor(np.ones((1, P), np.float32), name="ones_const")
        ones_row = const.tile([1, P], F32R, name="ones_row")
        nc.sync.dma_start(ones_row[:], ones_dram.ap().bitcast(F32R))
        bq_sb = const.tile([P, 3 * D // P], F32, name="bq_sb")
        nc.sync.dma_start(bq_sb[:], b_qkv.ap().rearrange("(o p) -> p o", p=P))
        bv_sb = const.tile([1, D], F32R, name="bv_sb")
        nc.sync.dma_start(bv_sb[:], b_qkv.ap()[None, 2 * D:3 * D].bitcast(F32R))
        b1_sb = const.tile([P, NFT], F32, name="b1_sb")
        nc.sync.dma_start(b1_sb[:], b_ff1.ap().rearrange("(o p) -> p o", p=P))
        b2_sb = const.tile([1, D], F32R, name="b2_sb")
        nc.sync.dma_start(b2_sb[:], b_ff2.ap()[None, :].bitcast(F32R))

        # Tiny collective issued immediately: absorbs the rank barrier and
        # ncfw warm-up (~30us) concurrently with the projection phase so the
        # real AllGathers start with only per-op latency.
        warm_in = dramp.tile([64], BF16, name="warm_in")
        warm_out = dramp.tile([R * 64], BF16, addr_space="Shared",
                              name="warm_out")
        nc.vector.memset(warm_sb := const.tile([1, 64], BF16, name="warm_sb"), 0.0)
        nc.scalar.dma_start(warm_in[:].rearrange("(a b) -> a b", a=1), warm_sb[:])
        nc.gpsimd.collective_compute(
            "AllGather", mybir.AluOpType.bypass,
            replica_groups=[list(range(R))],
            ins=[warm_in[:]], outs=[warm_out[:]],
        )

        ag_ins, ag_outs = [], []
        for gi, grp in enumerate(AG_GROUPS):
            n = len(grp) * PAIR_ELEMS
            ag_ins.append(dramp.tile([n], BF16, name=f"ag_in{gi}"))
            ag_outs.append(dramp.tile([R * n], BF16, addr_space="Shared",
                                      name=f"ag_out{gi}"))

        attnT = [persist.tile([P, TL], BF16, name=f"attnT{i}") for i in range(NDT)]
        hTp = top.enter_context(tc.tile_pool(name="hTp", bufs=1))
        hT = [hTp.tile([P, TL], BF16, name=f"hT{f}") for f in range(NFT)]
        # FFN weight pools at top scope so their loads can prefetch during
        # the attention phase.
        w1p = top.enter_context(tc.tile_pool(name="w1p", bufs=3))
        w2p = top.enter_context(tc.tile_pool(name="w2p", bufs=4))

        kv_scope = top.enter_context(ExitStack())
        kvp = kv_scope.enter_context(tc.tile_pool(name="kvp", bufs=1))
        qT = [kvp.tile([P, TL], BF16, name=f"qT{p}") for p in range(NPAIR)]
        kT_loc = [kvp.tile([P, TL], BF16, name=f"kTl{p}") for p in range(NPAIR)]
        # padded v in two half-tiles per t-tile (pairs 0-2 | pairs 3-5) so
        # group-0 payload DMAs don't wait on the second half's projection
        v_half = [[kvp.tile([P, 3 * PW], BF16, name=f"vp{t}_{h}")
                   for h in range(2)] for t in range(NTT)]
        kTf = [[kvp.tile([P, TL], BF16, name=f"kTf{p}_{r}") for r in range(R)]
               for p in range(NPAIR)]
        vf = [[kvp.tile([P, NTT * PW], BF16, name=f"vf{p}_{r}") for r in range(R)]
              for p in range(NPAIR)]

        # ------------------------------------------------------------------
        # Phase 1: x -> x^T, QKV projections, pipelined k/v AllGathers
        # ------------------------------------------------------------------
        with ExitStack() as ph1:
            xp = ph1.enter_context(tc.tile_pool(name="xp", bufs=1))
            wqp = ph1.enter_context(tc.tile_pool(name="wqp", bufs=3))
            psT = ph1.enter_context(tc.tile_pool(name="psT", bufs=2, space="PSUM"))
            psQ = ph1.enter_context(tc.tile_pool(name="psQ", bufs=2, space="PSUM"))

            x_sb = [xp.tile([P, D], F32, name=f"x{t}") for t in range(NTT)]
            for t in range(NTT):
                nc.sync.dma_start(x_sb[t][:], x.ap()[P * t:P * (t + 1), :])

            xT = [xp.tile([P, TL], BF16, name=f"xT{d}") for d in range(NDT)]
            for dt_ in range(NDT):
                for tt in range(NTT):
                    pst = psT.tile([P, P], F32, tag="pst", name="pst")
                    nc.tensor.transpose(
                        pst[:], x_sb[tt][:, P * dt_:P * (dt_ + 1)], ident[:])
                    if (dt_ * NTT + tt) % 2 == 0:
                        nc.vector.tensor_copy(
                            xT[dt_][:, P * tt:P * (tt + 1)], pst[:])
                    else:
                        nc.scalar.copy(
                            xT[dt_][:, P * tt:P * (tt + 1)], pst[:])

            w_qkv_v = w_qkv.ap().rearrange("(o p) j -> p o j", p=P)

            def proj_jt(jt, out_tile, add_engine, dma_engine):
                """qkv^T tile for channel block jt: out[j, t] = W[:,j]^T x^T + b."""
                wq = wqp.tile([P, NDT, P], BF16, tag="wq", name="wq")
                dma_engine.dma_start(wq[:], w_qkv_v[:, :, P * jt:P * (jt + 1)])
                ps = psQ.tile([P, TL], F32, tag="psq", name="psq")
                for d_ in range(NDT):
                    nc.tensor.matmul(ps[:], wq[:, d_, :], xT[d_][:],
                                     start=(d_ == 0), stop=(d_ == NDT - 1))
                if add_engine == "vector":
                    nc.vector.tensor_scalar_add(out_tile[:], ps[:],
                                                bq_sb[:, jt:jt + 1])
                else:
                    nc.scalar.activation(out_tile[:], ps[:], AFT.Identity,
                                         bias=bq_sb[:, jt:jt + 1])

            def proj_v_half(o2):
                """v rows for heads [6*o2, 6*o2+6) into padded v_half tiles."""
                sl = slice(384 * o2, 384 * (o2 + 1))
                for tt in range(NTT):
                    ps = psQ.tile([P, TL], F32, tag="psq", name="psq")
                    for d_ in range(NDT):
                        nc.tensor.matmul(ps[:, :384],
                                         xT[d_][:, P * tt:P * (tt + 1)],
                                         wv[:, d_, sl],
                                         start=(d_ == 0), stop=False)
                    nc.tensor.matmul(ps[:, :384], ones_row[:], bv_sb[:, sl],
                                     start=False, stop=True)
                    vdst = v_half[tt][o2].rearrange("p (h e) -> p h e", e=HD + 1)
                    nc.vector.tensor_copy(
                        vdst[:, :, 0:HD],
                        ps[:, :384].rearrange("p (h e) -> p h e", e=HD))
                    nc.vector.memset(vdst[:, :, HD:HD + 1], 1.0)

            def stage_pair(gi, pi, p_):
                """Copy pair p_'s k/v payload into group gi's AG input."""
                off = pi * PAIR_ELEMS
                ag_k = ag_ins[gi][off:off + K_ELEMS].rearrange(
                    "(a b) -> a b", b=TL)
                nc.scalar.dma_start(ag_k[:, :], kT_loc[p_][:])
                ag_v = ag_ins[gi][off + K_ELEMS:off + PAIR_ELEMS].rearrange(
                    "(t c) -> t c", c=PW)
                half, pp = divmod(p_, 3)
                for tt in range(NTT):
                    nc.scalar.dma_start(
                        ag_v[P * tt:P * (tt + 1), :],
                        v_half[tt][half][:, PW * pp:PW * (pp + 1)])

            def kick(gi):
                nc.gpsimd.collective_compute(
                    "AllGather", mybir.AluOpType.bypass,
                    replica_groups=[list(range(R))],
                    ins=[ag_ins[gi][:]], outs=[ag_outs[gi][:]],
                )

            wv = xp.tile([P, NDT, D], BF16, name="wv")
            nc.scalar.dma_start(wv[:], w_qkv_v[:, :, 2 * D:3 * D])

            # group 0 (pair 0) as early as possible
            proj_jt(NDT + 0, kT_loc[0], "scalar", nc.sync)
            proj_v_half(0)
            stage_pair(0, 0, 0)
            kick(0)
            # group 1 (pairs 1-2)
            for p_ in (1, 2):
                proj_jt(NDT + p_, kT_loc[p_], "scalar", nc.sync)
                stage_pair(1, p_ - 1, p_)
            kick(1)
            # group 2 (pairs 3-5)
            proj_v_half(1)
            for p_ in (3, 4, 5):
                proj_jt(NDT + p_, kT_loc[p_], "scalar", nc.sync)
                stage_pair(2, p_ - 3, p_)
            kick(2)

            # q projections overlap the collectives
            for p_ in range(NPAIR):
                proj_jt(p_, qT[p_], "vector", nc.sync)

            # keep-warm filler: the PE would otherwise idle ~10-15us waiting
            # for the first AllGather, dropping the HAM clock gate back to
            # 1.2 GHz right as attention starts. Outputs are never read.
            for wi in range(28):
                psw = psQ.tile([P, TL], F32, tag="psq", name="psw")
                nc.tensor.matmul(psw[:], kT_loc[0][:, 0:P], xT[wi % NDT][:],
                                 start=True, stop=True)

            # AllGather returns: per (pair, rank) tiles so attention chunks
            # only wait on the slice they read; k and v on separate DMA rings.
            for gi, grp in enumerate(AG_GROUPS):
                n = len(grp) * PAIR_ELEMS
                ago = ag_outs[gi].rearrange("(r e) -> r e", e=n)
                for pi, p_ in enumerate(grp):
                    off = pi * PAIR_ELEMS
                    for r in range(R):
                        src_k = ago[r, off:off + K_ELEMS].rearrange(
                            "(a b) -> a b", b=TL)
                        nc.sync.dma_start(kTf[p_][r][:], src_k)
                        src_v = ago[r, off + K_ELEMS:off + PAIR_ELEMS].rearrange(
                            "(s pi2 c) -> pi2 s c", pi2=P, c=PW)
                        dst_v = vf[p_][r].rearrange("p (s c) -> p s c", c=PW)
                        nc.gpsimd.dma_start(dst_v[:], src_v)

        # ------------------------------------------------------------------
        # Phase 2: attention, one head pair at a time
        # ------------------------------------------------------------------
        with ExitStack() as ph2:
            scp = ph2.enter_context(tc.tile_pool(name="scp", bufs=3, space="PSUM"))
            accp = ph2.enter_context(tc.tile_pool(name="accp", bufs=2, space="PSUM"))
            weip = ph2.enter_context(tc.tile_pool(name="weip", bufs=4))
            tailp = ph2.enter_context(tc.tile_pool(name="tailp", bufs=2))

            for p_ in range(NPAIR):
                acc0 = accp.tile([HD + 1, TL], F32, tag="acc", name="acc0")
                acc1 = accp.tile([HD + 1, TL], F32, tag="acc", name="acc1")
                for c in range(R * NTT):
                    r, s = divmod(c, NTT)
                    kt = kTf[p_][r]
                    sc = scp.tile([P, 2 * TL], F32, tag="sc", name="sc")
                    nc.tensor.matmul(sc[:, 0:TL],
                                     kt[0:HD, P * s:P * (s + 1)],
                                     qT[p_][0:HD, :], start=True, stop=True)
                    nc.tensor.matmul(sc[:, TL:2 * TL],
                                     kt[HD:P, P * s:P * (s + 1)],
                                     qT[p_][HD:P, :], start=True, stop=True)
                    wei = weip.tile([P, 2 * TL], BF16, tag="wei", name="wei")
                    if c % 7 in EXP_DVE_RESIDUES:
                        nc.vector._custom_dve(EXP3, out=wei[:], in0=sc[:],
                                              s0=EXP_K1, s1=EXP_K2, imm2=EXP_K3)
                    else:
                        nc.scalar.activation(wei[:], sc[:], AFT.Exp)
                    vt = vf[p_][r]
                    nc.tensor.matmul(acc0[:],
                                     vt[:, PW * s:PW * s + HD + 1],
                                     wei[:, 0:TL],
                                     start=(c == 0), stop=(c == R * NTT - 1))
                    nc.tensor.matmul(acc1[:],
                                     vt[:, PW * s + HD + 1:PW * (s + 1)],
                                     wei[:, TL:2 * TL],
                                     start=(c == 0), stop=(c == R * NTT - 1))

                # softmax tail: den -> 1/den -> broadcast -> scale
                from concourse.dve_ops import (
                    RECIPROCAL_APPROX_FAST, RECIPROCAL_APPROX_NR,
                    RECIP_APPROX_FAST_CONSTS as _RC)
                for hh, acc in ((0, acc0), (1, acc1)):
                    # custom-DVE ops require equal partition bases on all
                    # operands; stage the denominator row at partition 0.
                    denc = tailp.tile([1, TL], F32, tag="denc", name="denc")
                    nc.vector.tensor_copy(denc[:], acc[HD:HD + 1, :])
                    den = tailp.tile([1, TL], F32R, tag="den", name="den")
                    scr = tailp.tile([1, TL], F32, tag="scr", name="scr")
                    nc.vector._custom_dve(
                        RECIPROCAL_APPROX_FAST, out=scr[:],
                        in0=denc[:], s0=_RC["s0"], s1=_RC["s1"],
                        imm2=_RC["imm2"])
                    nc.vector._custom_dve(
                        RECIPROCAL_APPROX_NR, out=den[:],
                        in0=denc[:], in1=scr[:], s0=2.0)
                    bc = scp.tile([HD, TL], F32, tag="sc", name="bc")
                    nc.tensor.matmul(bc[:], ones_row[:, 0:HD], den[:],
                                     start=True, stop=True)
                    recb = tailp.tile([HD, TL], F32, tag="recb", name="recb")
                    nc.vector.tensor_copy(recb[:], bc[:])
                    nc.vector.tensor_tensor(
                        attnT[p_][HD * hh:HD * (hh + 1), :], acc[0:HD, :],
                        recb[:], mybir.AluOpType.mult)

        kv_scope.close()

        # ------------------------------------------------------------------
        # Phase 3a: FFN1  h^T[f, t] = gelu(W1^T attn^T + b1)
        # ------------------------------------------------------------------
        w_ff1_v = w_ff1.ap().rearrange("(o p) f -> p o f", p=P)
        with ExitStack() as ph3:
            ps1 = ph3.enter_context(tc.tile_pool(name="ps1", bufs=2, space="PSUM"))
            for ft in range(NFT):
                w1 = w1p.tile([P, NDT, P], BF16, tag="w1", name="w1")
                nc.gpsimd.dma_start(w1[:], w_ff1_v[:, :, P * ft:P * (ft + 1)])
                ps = ps1.tile([P, TL], F32, tag="ps1t", name="ps1t")
                for d_ in range(NDT):
                    nc.tensor.matmul(ps[:], w1[:, d_, :], attnT[d_][:],
                                     start=(d_ == 0), stop=(d_ == NDT - 1))
                nc.scalar.activation(hT[ft][:], ps[:], AFT.Gelu,
                                     bias=b1_sb[:, ft:ft + 1])

        # ------------------------------------------------------------------
        # Phase 3b: FFN2  out[t, o] = h^T^T W2 + b2
        # ------------------------------------------------------------------
        w_ff2_v = w_ff2.ap().rearrange("(o p) d -> p o d", p=P)
        with ExitStack() as ph4:
            ps2 = ph4.enter_context(tc.tile_pool(name="ps2", bufs=1, space="PSUM"))
            outp = ph4.enter_context(tc.tile_pool(name="outp", bufs=1))
            acc2 = [ps2.tile([P, 384], F32, name=f"acc2_{g}") for g in range(8)]
            for ft in range(NFT):
                w2 = w2p.tile([P, D], BF16, tag="w2", name="w2")
                nc.gpsimd.dma_start(w2[:], w_ff2_v[:, ft, :])
                for tt in range(NTT):
                    for o2 in range(2):
                        g = tt * 2 + o2
                        nc.tensor.matmul(acc2[g][:],
                                         hT[ft][:, P * tt:P * (tt + 1)],
                                         w2[:, 384 * o2:384 * (o2 + 1)],
                                         start=(ft == 0), stop=False)
            out_sb = [outp.tile([P, D], F32, name=f"out{tt}") for tt in range(NTT)]
            for tt in range(NTT):
                for o2 in range(2):
                    g = tt * 2 + o2
                    sl = slice(384 * o2, 384 * (o2 + 1))
                    nc.tensor.matmul(acc2[g][:], ones_row[:], b2_sb[:, sl],
                                     start=False, stop=True)
                    nc.vector.tensor_copy(out_sb[tt][:, sl], acc2[g][:])
                nc.scalar.dma_start(y.ap()[P * tt:P * (tt + 1), :], out_sb[tt][:])

    # The Tile path never runs bacc's codegen_inst_isa_subclasses pass, so
    # custom-DVE ISA wrappers would serialize with empty instruction bytes
    # ("ISA wrong length" in walrus). Lower them in place here.
    import concourse.bass_isa as bass_isa
    for func in nc.m.functions:
        for blk in func.blocks:
            i = 0
            while i < len(blk.instructions):
                inst = blk.instructions[i]
                if isinstance(inst, bass_isa.InstCustomDveAnt):
                    lowered = mybir.codegen_inst_isa_one(inst, nc._state, nc.isa)
                    assert isinstance(lowered, list)
                    del nc.inst_map[inst.name]
                    blk.instructions[i:i + 1] = lowered
                    for li in lowered:
                        nc.inst_map[li.name] = li
                    i += len(lowered)
                else:
                    i += 1

    return nc


def _get_nc():
    if "nc" not in _NC_CACHE:
        _NC_CACHE["nc"] = _build_nc()
    return _NC_CACHE["nc"]


def run_sharded(inputs, **run_kwargs):
    """Run the SPMD kernel; returns (full_output [1,4096,768], BassKernelResults)."""
    x = np.ascontiguousarray(np.asarray(inputs["x"], dtype=np.float32))
    assert x.shape == (1, T, D), x.shape

    w_qkv = np.asarray(inputs["w_qkv"], dtype=np.float32).copy()
    b_qkv = np.asarray(inputs["b_qkv"], dtype=np.float32).copy()
    # fold the 1/sqrt(d) score scale into the q projection
    w_qkv[:, 0:D] *= SCALE
    b_qkv[0:D] *= SCALE

    common = {
        "w_qkv": np.ascontiguousarray(w_qkv.astype(ml_dtypes.bfloat16)),
        "b_qkv": np.ascontiguousarray(b_qkv),
        "w_ff1": np.ascontiguousarray(
            np.asarray(inputs["w_ff1"], dtype=np.float32).astype(ml_dtypes.bfloat16)),
        "b_ff1": np.ascontiguousarray(np.asarray(inputs["b_ff1"], dtype=np.float32)),
        "w_ff2": np.ascontiguousarray(
            np.asarray(inputs["w_ff2"], dtype=np.float32).astype(ml_dtypes.bfloat16)),
        "b_ff2": np.ascontiguousarray(np.asarray(inputs["b_ff2"], dtype=np.float32)),
    }
    in_maps = []
    for r in range(R):
        m = dict(common)
        m["x"] = np.ascontiguousarray(x[0, TL * r:TL * (r + 1), :])
        in_maps.append(m)
    nc = _get_nc()
    res = run_bass_kernel_spmd(nc, in_maps, core_ids=list(range(R)), **run_kwargs)
    out = np.concatenate([res.results[r]["y"] for r in range(R)], axis=0)
    return out.reshape(1, T, D), res


def kernel(**inputs):
    out, _ = run_sharded(inputs)
    return out
